# revision 7
# baseline (speedup 1.0000x reference)
"""Trainium2 Bass kernel for GQA causal attention (B=2, T=4096, 8 q-heads,
2 kv-heads, head_dim=64, RoPE) with QKV/O projections.

Sharding: 16 (batch, head) pairs over 8 cores -> each core handles one batch
and one pair of query heads (which share a single KV head, rep=4). The o_proj
is row-sharded over head pairs; per-core partial outputs are summed on host.

Per-core device kernel (all matmuls fp32r = full-rate single-pass fp32):
  xT [512,4096] -> qT [128,4096] (2 heads, RoPE), kT [64,4096] (RoPE),
  v_aug [128,32,65] (v tiles + ones column).
  For each 512-wide causal q-block: scores^T tiles [128k,512q] on PE,
  exp on ACT (softmax scale folded in, no max-subtraction: scores are
  bounded ~N(0,1) after the 1/sqrt(d) scale), causal masking by 0/1
  multiplicative mask on the diagonal band, then AV via out^T[65,512]
  accumulation where row 64 (ones column of v_aug) yields the softmax
  denominator for free. Normalize with reciprocal+broadcast, o_proj
  as two K=64 matmuls against the row slice of Wo.
"""

import numpy as np

N_EMBD = 512
N_HEADS = 8
N_KV_HEADS = 2
HD = 64
B = 2
T = 4096
ROPE_BASE = 1000000.0
N_CORES = 8
SCALE = HD ** -0.5
TB = T // 512   # 8 query blocks of 512
KT = T // 128   # 32 key tiles of 128

_CACHE = {}


def _build_program():
    import concourse.bacc as bacc
    import concourse.mybir as mybir
    import concourse.tile as tile

    f32 = mybir.dt.float32
    f32r = mybir.dt.float32r
    MULT = mybir.AluOpType.mult
    ADD = mybir.AluOpType.add
    EXP = mybir.ActivationFunctionType.Exp

    nc = bacc.Bacc("TRN2", target_bir_lowering=False)

    xT_h = nc.dram_tensor("xt_in", [N_EMBD, T], f32, kind="ExternalInput")
    wq_h = nc.dram_tensor("wq_in", [N_EMBD, 128], f32, kind="ExternalInput")
    wqr_h = nc.dram_tensor("wqr_in", [N_EMBD, 128], f32, kind="ExternalInput")
    wk_h = nc.dram_tensor("wk_in", [N_EMBD, 64], f32, kind="ExternalInput")
    wkr_h = nc.dram_tensor("wkr_in", [N_EMBD, 64], f32, kind="ExternalInput")
    wv_h = nc.dram_tensor("wv_in", [N_EMBD, 64], f32, kind="ExternalInput")
    woa_h = nc.dram_tensor("woa_in", [64, N_EMBD], f32, kind="ExternalInput")
    wob_h = nc.dram_tensor("wob_in", [64, N_EMBD], f32, kind="ExternalInput")
    cos2_h = nc.dram_tensor("cos2_in", [128, T], f32, kind="ExternalInput")
    sin2_h = nc.dram_tensor("sin2_in", [128, T], f32, kind="ExternalInput")
    mask_h = nc.dram_tensor("mask_in", [128, 896], f32, kind="ExternalInput")
    y_h = nc.dram_tensor("y_out", [T, N_EMBD], f32, kind="ExternalOutput")

    xT_r = xT_h[:].rearrange("(ko p) t -> p ko t", p=128)

    def r(ap):
        return ap.bitcast(f32r)

    with tile.TileContext(nc) as tc:
        with tc.tile_pool(name="persist", bufs=1) as pp:
            qT = pp.tile([128, T], f32, name="qT")
            kT = pp.tile([64, T], f32, name="kT")
            vaug = pp.tile([128, KT, 65], f32, name="vaug")
            mask_sb = pp.tile([128, 896], f32, name="mask_sb")
            woa_sb = pp.tile([64, N_EMBD], f32, name="woa_sb")
            wob_sb = pp.tile([64, N_EMBD], f32, name="wob_sb")
            nc.sync.dma_start(mask_sb[:], mask_h[:])
            nc.sync.dma_start(r(woa_sb[:]), r(woa_h[:]))
            nc.sync.dma_start(r(wob_sb[:]), r(wob_h[:]))
            ones_sb = pp.tile([128, 1], f32, name="ones_sb")
            nc.vector.memset(ones_sb[:], 1.0)
            nc.vector.tensor_copy(r(vaug[:, :, 64:65]), ones_sb[:, None, :].to_broadcast((128, KT, 1)))

            # ---------------- phase 1: projections + RoPE ----------------
            with tc.tile_pool(name="wpool", bufs=1) as wp, \
                 tc.tile_pool(name="xpool", bufs=3) as xp, \
                 tc.tile_pool(name="ropep", bufs=2) as rp, \
                 tc.tile_pool(name="qkps", bufs=1, space="PSUM") as qkps, \
                 tc.tile_pool(name="vps", bufs=2, space="PSUM") as vps:

                wq_sb = wp.tile([128, 4, 128], f32, name="wq_sb")
                wqr_sb = wp.tile([128, 4, 128], f32, name="wqr_sb")
                wk_sb = wp.tile([128, 4, 64], f32, name="wk_sb")
                wkr_sb = wp.tile([128, 4, 64], f32, name="wkr_sb")
                wv_sb = wp.tile([128, 4, 64], f32, name="wv_sb")
                cos2_sb = wp.tile([128, T], f32, name="cos2_sb")
                sin2_sb = wp.tile([128, T], f32, name="sin2_sb")
                nc.sync.dma_start(r(wq_sb[:]), r(wq_h[:].rearrange("(ko p) m -> p ko m", p=128)))
                nc.sync.dma_start(r(wqr_sb[:]), r(wqr_h[:].rearrange("(ko p) m -> p ko m", p=128)))
                nc.sync.dma_start(r(wk_sb[:]), r(wk_h[:].rearrange("(ko p) m -> p ko m", p=128)))
                nc.sync.dma_start(r(wkr_sb[:]), r(wkr_h[:].rearrange("(ko p) m -> p ko m", p=128)))
                nc.sync.dma_start(r(wv_sb[:]), r(wv_h[:].rearrange("(ko p) m -> p ko m", p=128)))
                nc.sync.dma_start(cos2_sb[:], cos2_h[:])
                nc.sync.dma_start(sin2_sb[:], sin2_h[:])

                for tb in range(TB):
                    sl = slice(tb * 512, (tb + 1) * 512)
                    xt = xp.tile([128, 4, 512], f32, name=f"xt{tb}", tag="xt")
                    nc.sync.dma_start(r(xt[:]), r(xT_r[:, :, sl]))

                    q_ps = qkps.tile([128, 512], f32, name=f"qps{tb}", tag="q")
                    qr_ps = qkps.tile([128, 512], f32, name=f"qrps{tb}", tag="qr")
                    k_ps = qkps.tile([64, 512], f32, name=f"kps{tb}", tag="k")
                    kr_ps = qkps.tile([64, 512], f32, name=f"krps{tb}", tag="kr")
                    for ks in range(4):
                        st, sp = (ks == 0), (ks == 3)
                        nc.tensor.matmul(q_ps[:], r(wq_sb[:, ks, :]), r(xt[:, ks, :]), start=st, stop=sp)
                        nc.tensor.matmul(qr_ps[:], r(wqr_sb[:, ks, :]), r(xt[:, ks, :]), start=st, stop=sp)
                        nc.tensor.matmul(k_ps[:], r(wk_sb[:, ks, :]), r(xt[:, ks, :]), start=st, stop=sp)
                        nc.tensor.matmul(kr_ps[:], r(wkr_sb[:, ks, :]), r(xt[:, ks, :]), start=st, stop=sp)

                    tmp = rp.tile([128, 512], f32, name=f"tmp{tb}", tag="tmp")
                    nc.vector.tensor_tensor(r(qT[:, sl]), q_ps[:], cos2_sb[:, sl], MULT)
                    nc.vector.tensor_tensor(tmp[:], qr_ps[:], sin2_sb[:, sl], MULT)
                    nc.vector.tensor_tensor(r(qT[:, sl]), qT[:, sl], tmp[:], ADD)
                    tmpk = rp.tile([64, 512], f32, name=f"tmpk{tb}", tag="tmpk")
                    nc.vector.tensor_tensor(r(kT[:, sl]), k_ps[:], cos2_sb[0:64, sl], MULT)
                    nc.vector.tensor_tensor(tmpk[:], kr_ps[:], sin2_sb[0:64, sl], MULT)
                    nc.vector.tensor_tensor(r(kT[:, sl]), kT[:, sl], tmpk[:], ADD)

                    for ktl in range(4):
                        kt = 4 * tb + ktl
                        v_ps = vps.tile([128, 64], f32, name=f"vps{kt}", tag="v")
                        for ks in range(4):
                            nc.tensor.matmul(
                                v_ps[:],
                                r(xt[:, ks, ktl * 128:(ktl + 1) * 128]),
                                r(wv_sb[:, ks, :]),
                                start=(ks == 0), stop=(ks == 3),
                            )
                        nc.vector.tensor_copy(r(vaug[:, kt, 0:64]), v_ps[:])

            # ---------------- phase 2: attention + o_proj ----------------
            with tc.tile_pool(name="attp", bufs=1) as ap_, \
                 tc.tile_pool(name="ptp", bufs=3) as ptp, \
                 tc.tile_pool(name="dramp", bufs=2, space="DRAM") as dramp, \
                 tc.tile_pool(name="sps", bufs=2, space="PSUM") as sps, \
                 tc.tile_pool(name="ops", bufs=2, space="PSUM") as ops, \
                 tc.tile_pool(name="yps", bufs=2, space="PSUM") as yps:

                for qb in range(TB):
                    qsl = slice(qb * 512, (qb + 1) * 512)
                    G = 4 * qb + 4
                    qbB = ap_.tile([64, 512], f32, name=f"qbB{qb}", tag="qbB", bufs=2)
                    nc.sync.dma_start(r(qbB[:]), r(qT[64:128, qsl]))
                    o2 = []
                    for h in range(2):
                        qrhs = qT[0:64, qsl] if h == 0 else qbB[:]
                        o_ps = ops.tile([65, 512], f32, name=f"ops{qb}_{h}", tag="o")
                        for kp in range(G // 2):
                            s_ps = sps.tile([128, 1024], f32, name=f"sps{qb}_{h}_{kp}", tag="s")
                            for half in range(2):
                                kt = 2 * kp + half
                                nc.tensor.matmul(
                                    s_ps[:, half * 512:(half + 1) * 512],
                                    r(kT[:, kt * 128:(kt + 1) * 128]),
                                    r(qrhs),
                                    start=True, stop=True,
                                )
                            pt = ptp.tile([128, 1024], f32, name=f"pt{qb}_{h}_{kp}", tag="pt")
                            nc.scalar.activation(r(pt[:]), s_ps[:], EXP, scale=SCALE)
                            for half in range(2):
                                kt = 2 * kp + half
                                j = kt - 4 * qb
                                psl = slice(half * 512, (half + 1) * 512)
                                if j >= 0:
                                    msl = mask_sb[:, (3 - j) * 128:(3 - j) * 128 + 512]
                                    nc.vector.tensor_tensor(r(pt[:, psl]), pt[:, psl], msl, MULT)
                                nc.tensor.matmul(
                                    o_ps[:],
                                    r(vaug[:, kt, :]),
                                    r(pt[:, psl]),
                                    start=(kt == 0), stop=(kt == G - 1),
                                )
                        rec = ap_.tile([65, 512], f32, name=f"rec{qb}_{h}", tag="rec", bufs=2)
                        nc.vector.reciprocal(rec[64:65, :], o_ps[64:65, :])
                        recd = dramp.tile([1, 512], f32, name=f"recd{qb}_{h}", tag="recd", bufs=2)
                        nc.sync.dma_start(recd[:], rec[64:65, :])
                        recb = ap_.tile([64, 512], f32, name=f"recb{qb}_{h}", tag="recb", bufs=2)
                        nc.sync.dma_start(recb[:], recd[:].to_broadcast((64, 512)))
                        o2t = ap_.tile([64, 512], f32, name=f"o2_{qb}_{h}", tag=f"o2{h}", bufs=2)
                        nc.vector.tensor_tensor(r(o2t[:]), o_ps[0:64, :], recb[:], MULT)
                        o2.append(o2t)

                    for tl in range(4):
                        tsl = slice(tl * 128, (tl + 1) * 128)
                        y_ps = yps.tile([128, 512], f32, name=f"yps{qb}_{tl}", tag="y")
                        nc.tensor.matmul(y_ps[:], r(o2[0][:, tsl]), r(woa_sb[:]), start=True, stop=False)
                        nc.tensor.matmul(y_ps[:], r(o2[1][:, tsl]), r(wob_sb[:]), start=False, stop=True)
                        y_sb = ap_.tile([128, 512], f32, name=f"ysb{qb}_{tl}", tag="ysb", bufs=3)
                        nc.vector.tensor_copy(y_sb[:], y_ps[:])
                        nc.sync.dma_start(y_h[:][qb * 512 + tl * 128:qb * 512 + (tl + 1) * 128, :], y_sb[:])

    nc.finalize()
    return nc


def _get_program():
    if "nc" not in _CACHE:
        _CACHE["nc"] = _build_program()
    return _CACHE["nc"]


def _rot_w(w):
    # rotate_half applied to the per-head output columns of a projection
    # weight: rot(q) = W_rot^T x with W_rot[:, :32] = -W[:, 32:64] etc.
    w3 = w.reshape(w.shape[0], -1, 64)
    wr = np.concatenate([-w3[:, :, 32:], w3[:, :, :32]], axis=-1)
    return np.ascontiguousarray(wr.reshape(w.shape), dtype=np.float32)


def _host_tables():
    if "tables" in _CACHE:
        return _CACHE["tables"]
    inv = (1.0 / (ROPE_BASE ** (np.arange(0, HD, 2, dtype=np.float64) / HD))).astype(np.float32)
    t = np.arange(T, dtype=np.float32)
    freqs = np.outer(t, inv).astype(np.float32)          # [T, 32]
    emb = np.concatenate([freqs, freqs], axis=-1)        # [T, 64]
    cosT = np.ascontiguousarray(np.cos(emb).astype(np.float32).T)  # [64, T]
    sinT = np.ascontiguousarray(np.sin(emb).astype(np.float32).T)
    cos2 = np.ascontiguousarray(np.concatenate([cosT, cosT], axis=0))  # [128, T]
    sin2 = np.ascontiguousarray(np.concatenate([sinT, sinT], axis=0))
    p = np.arange(128, dtype=np.int64)[:, None]
    m = np.arange(896, dtype=np.int64)[None, :]
    mask01 = (m >= p + 384).astype(np.float32)           # [128, 896]
    _CACHE["tables"] = (cos2, sin2, mask01)
    return _CACHE["tables"]


def _make_in_maps(x, Wq, bq, Wk, bk, Wv, bv, Wo):
    assert not np.any(bq) and not np.any(bk) and not np.any(bv), \
        "nonzero qkv biases not supported by this kernel build"
    cos2, sin2, mask01 = _host_tables()
    x = np.asarray(x, dtype=np.float32)
    xTs = [np.ascontiguousarray(x[b].T) for b in range(B)]  # [512, T] each
    in_maps = []
    for c in range(N_CORES):
        b, pr = c // 4, c % 4
        kv = pr // 2
        wq_c = np.ascontiguousarray(Wq[:, pr * 128:(pr + 1) * 128], dtype=np.float32)
        wk_c = np.ascontiguousarray(Wk[:, kv * 64:(kv + 1) * 64], dtype=np.float32)
        wv_c = np.ascontiguousarray(Wv[:, kv * 64:(kv + 1) * 64], dtype=np.float32)
        wo_c = np.asarray(Wo[pr * 128:(pr + 1) * 128, :], dtype=np.float32)
        in_maps.append({
            "xt_in": xTs[b],
            "wq_in": wq_c,
            "wqr_in": _rot_w(wq_c),
            "wk_in": wk_c,
            "wkr_in": _rot_w(wk_c),
            "wv_in": wv_c,
            "woa_in": np.ascontiguousarray(wo_c[0:64, :]),
            "wob_in": np.ascontiguousarray(wo_c[64:128, :]),
            "cos2_in": cos2,
            "sin2_in": sin2,
            "mask_in": mask01,
        })
    return in_maps


def _run(in_maps, trace=False):
    from concourse.bass_utils import run_bass_kernel_spmd
    nc = _get_program()
    return run_bass_kernel_spmd(nc, in_maps, list(range(N_CORES)), trace=trace)


def kernel(x, Wq, bq, Wk, bk, Wv, bv, Wo, _trace=False):
    in_maps = _make_in_maps(x, Wq, bq, Wk, bk, Wv, bv, Wo)
    res = _run(in_maps, trace=_trace)
    y = np.zeros((B, T, N_EMBD), dtype=np.float32)
    for c in range(N_CORES):
        y[c // 4] += res.results[c]["y_out"]
    if _trace:
        _CACHE["last_exec_time_ns"] = res.exec_time_ns
    return y
